# revision 32
# baseline (speedup 1.0000x reference)
"""Trainium2 Bass kernel for nn_CapsuleLayer_9852654977072.

The reference module collapses mathematically: the routing loop's coupling
logits `b` stay zero (faithfully-reproduced bug in the original torch code),
so routing coefficients are a fixed spatial map r(h,w) = 1/(8*cnt(h,w)) where
cnt is the 5x5 box-count inside the image. The whole module is therefore:

    p = conv2d(u as [N,64,H,W], Wd as [128,64,5,5], pad=2) * s(h,w)
    v = squash_z1(p)   # groups of 16 channels
    out[n,t1,z1,h,w] = v

Device strategy (8 cores, SPMD): shard (batch n in 0..3) x (row-half in 0..1).
Each core computes all 128 output channels for 64 rows of one image.

Conv: inputs shipped as XA/XC [128, 68, 132] f32 whose partition halves hold u
shifted by (+0row,+1row) and (+2row+0col,+2row+1col) respectively, columns
padded by 2. Per 4-row block, 13 PSUM-accumulated fp32r matmuls (N=512, full
PE rate, self-loading weights) cover all 25 taps: 10 XA row-pairs +
2 XC col-pairs + 1 K=64 single.

Squash: per group of blocks ([4,4,4,2,2]), block-diagonal matmuls pack
m2 = sum_z1 q^2 for all (block, t1) pairs into one [8*ng, 512] PSUM tile.
The factor F = y/((1+y)*sqrt(y_raw+eps)), y = s^2*y_raw runs ONCE per group
on ACT/DVE in bf16 (no GpSimd), then expand matmuls broadcast F back to the
128 channels and v = p * F (bf16, shipped bf16 over HBM and upcast on host).
Expansion of group g-1 is emitted after the conv of group g so the PE queue
never stalls on the factor chain. Dummy matmuls at kernel start keep the PE
busy during the DMA lead-in so HAM stays un-throttled.
"""

import numpy as np

T0, Z0, T1, Z1, KK, PAD = 4, 16, 8, 16, 5, 2
N, H, W_SP = 4, 128, 128
CIN, COUT = T0 * Z0, T1 * Z1  # 64, 128
N_CORES = 8
ROWS = 64          # output rows per core
XROWS = 68         # input rows incl. halo
XCOLS = 132        # 128 + 2*PAD
BLK = 4            # output rows per block
N_BLKS = ROWS // BLK   # 16
GROUPS = [(0, 4), (4, 4), (8, 4), (12, 2), (14, 2)]  # (first block, n blocks)

# conv matmul j -> (source, row_off, col_off); weights match in _weight_tiles
_MM_SLICES = (
    [('XA', dy + 2, dx + 2) for dy in (-2, 0) for dx in (-2, -1, 0, 1, 2)]
    + [('XC', 2, 0), ('XC', 2, 2), ('XC', 2, 4)]
)

_MM_ORDER = [0, 12, 1, 2, 3, 4, 5, 6, 7, 8, 9, 10, 11]

_CACHE = {}


def _bp_of(blk):
    for b0, ng in GROUPS:
        if b0 <= blk < b0 + ng:
            return blk - b0
    raise ValueError(blk)


def _weight_tiles(W):
    Wd = W.transpose(1, 0, 2, 3, 4).reshape(COUT, CIN, KK, KK)
    wl = np.zeros((128, 13, 128), np.float32)  # [k, j, m]
    j = 0
    for dy in (-2, 0):
        for dx in (-2, -1, 0, 1, 2):
            wl[0:64, j, :] = Wd[:, :, dy + 2, dx + 2].T
            wl[64:128, j, :] = Wd[:, :, dy + 3, dx + 2].T
            j += 1
    for dx0 in (-2, 0):
        wl[0:64, j, :] = Wd[:, :, 4, dx0 + 2].T
        wl[64:128, j, :] = Wd[:, :, 4, dx0 + 3].T
        j += 1
    wl[0:64, j, :] = Wd[:, :, 4, 4].T  # single tap (2,2) on lo partitions
    return wl


def _inputs_core(x, half):
    """x: [64, H, W] one image channel-major. Returns XA, XC [128, 68, 132]."""
    base = half * 64 - 2
    XA = np.zeros((128, XROWS, XCOLS), np.float32)
    XC = np.zeros((128, XROWS, XCOLS), np.float32)

    def fill(dst, roff, c0, c1):
        lo, hi = max(0, -(base + roff)), min(XROWS, H - base - roff)
        dst[:, lo:hi, c0:c1] = x[:, base + roff + lo:base + roff + hi, :]

    fill(XA[0:64], 0, 2, 130)
    fill(XA[64:128], 1, 2, 130)
    fill(XC[0:64], 2, 2, 130)
    fill(XC[64:128], 2, 1, 129)
    return XA, XC


def _s2_groups(half):
    """[32, len(GROUPS), BLK*128] f32: s^2 at partition m=8*bp+t1
    (t1-replicated), group g, flat pos = (row-within-block, col)."""
    idx = np.arange(H)
    cnt = (np.minimum(idx + 2, H - 1) - np.maximum(idx - 2, 0) + 1).astype(np.float64)
    s = 1.0 / (8.0 * cnt[:, None] * cnt[None, :])  # [H, W]
    s2 = (s * s)[half * 64:(half + 1) * 64, :]     # [64, 128]
    out = np.zeros((32, len(GROUPS), BLK * 128), np.float64)
    for g, (b0, ng) in enumerate(GROUPS):
        for bp in range(ng):
            blk = b0 + bp
            rows = s2[blk * BLK:(blk + 1) * BLK, :].reshape(-1)  # [512]
            out[8 * bp:8 * bp + 8, g, :] = rows[None, :]
    return np.ascontiguousarray(
        out.astype(np.float32).reshape(32, len(GROUPS) * BLK * 128))


def _bdv():
    """[128, N_BLKS*32]: c=(t1,z1) -> partition m=8*bp(blk)+t1, sum over z1."""
    bd = np.zeros((128, N_BLKS, 32), np.float32)
    c = np.arange(128)
    for blk in range(N_BLKS):
        bd[c, blk, 8 * _bp_of(blk) + c // 16] = 1.0
    return np.ascontiguousarray(bd.reshape(128, N_BLKS * 32))


def _exv():
    """[32, N_BLKS*128]: partition p=8*bp(blk)+t1 -> channels c, c//16==t1."""
    ex = np.zeros((32, N_BLKS, 128), np.float32)
    c = np.arange(128)
    for blk in range(N_BLKS):
        ex[8 * _bp_of(blk) + c // 16, blk, c] = 1.0
    return np.ascontiguousarray(ex.reshape(32, N_BLKS * 128))


def build_nc(reps=1):
    import concourse.bass as bass
    import concourse.bacc as bacc
    import concourse.mybir as mybir
    import concourse.tile as tile

    f32 = mybir.dt.float32
    f32r = mybir.dt.float32r
    bf16 = mybir.dt.bfloat16
    AF = mybir.ActivationFunctionType
    NG = len(GROUPS)

    nc = bacc.Bacc(None, target_bir_lowering=False)
    xa_d = nc.dram_tensor("xa", [128, XROWS * XCOLS], bf16, kind="ExternalInput")
    xc_d = nc.dram_tensor("xc", [128, XROWS * XCOLS], bf16, kind="ExternalInput")
    wl_d = nc.dram_tensor("wl", [128, 13 * 128], bf16, kind="ExternalInput")
    bdv_d = nc.dram_tensor("bdv", [128, N_BLKS * 32], bf16, kind="ExternalInput")
    exv_d = nc.dram_tensor("exv", [32, N_BLKS * 128], bf16, kind="ExternalInput")
    s2_d = nc.dram_tensor("s2", [32, NG * BLK * 128], f32, kind="ExternalInput")
    out_d = nc.dram_tensor("out", [128, ROWS * 128], bf16, kind="ExternalOutput")

    with tile.TileContext(nc) as tc:
        with (
            tc.tile_pool(name="consts", bufs=1) as consts,
            tc.tile_pool(name="sq", bufs=3) as sq,
            tc.tile_pool(name="psb", bufs=9) as psb,
            tc.tile_pool(name="fac", bufs=2) as fac,
            tc.tile_pool(name="ff", bufs=2) as ff,
            tc.tile_pool(name="vv", bufs=3) as vv,
            tc.tile_pool(name="pp", bufs=4, space="PSUM") as pp,
            tc.tile_pool(name="py", bufs=2, space="PSUM") as py,
            tc.tile_pool(name="pf", bufs=2, space="PSUM") as pf,
        ):
            # PE pre-warm: dummy matmuls on a zeroed tile keep the PE busy
            # during the input-DMA lead-in so HAM un-throttles to 2.4 GHz
            # before the first real conv matmul.
            dum = consts.tile([128, 512], f32)
            nc.gpsimd.memset(dum[:], 0.0)
            dum_ps = pf.tile([128, BLK, 128], f32, tag="fe")
            for _ in range(14):
                nc.tensor.matmul(dum_ps[:], dum[:, 0:128].bitcast(f32r),
                                 dum[:].bitcast(f32r), start=True, stop=True)

            wl_src = wl_d.ap().rearrange("p (j m) -> p j m", m=128)
            wlh = consts.tile([128, 2, 128], bf16)
            nc.sync.dma_start(out=wlh[:, 0:1, :], in_=wl_src[:, 0:1, :])
            nc.sync.dma_start(out=wlh[:, 1:2, :], in_=wl_src[:, 12:13, :])
            wlr = consts.tile([128, 11, 128], bf16)

            # Input chunks as separate tiles (deps are whole-tile): tile 0
            # holds block 0's rows, tile i (1..8) covers blocks {2i-1, 2i}.
            xa_src = xa_d.ap().rearrange("p (r c) -> p r c", c=XCOLS)
            xc_src = xc_d.ap().rearrange("p (r c) -> p r c", c=XCOLS)
            # blocks 0-3: per-block tiles (fast arrival); blocks 4+: pairs
            xat = [consts.tile([128, 6, XCOLS], bf16, name=f"xat{b}")
                   for b in range(4)] + [
                consts.tile([128, 10, XCOLS], bf16, name=f"xap{k}")
                for k in range(6)]
            xct = [consts.tile([128, 4, XCOLS], bf16, name=f"xct{b}")
                   for b in range(4)] + [
                consts.tile([128, 8, XCOLS], bf16, name=f"xcp{k}")
                for k in range(6)]
            nc.sync.dma_start(out=xat[0][:], in_=xa_src[:, 0:6, :])
            nc.sync.dma_start(out=xct[0][:], in_=xc_src[:, 2:6, :])
            nc.sync.dma_start(out=wlr, in_=wl_src[:, 1:12, :])
            for b in range(1, 4):
                nc.sync.dma_start(out=xat[b][:],
                                  in_=xa_src[:, 4 * b:4 * b + 6, :])
                nc.sync.dma_start(out=xct[b][:],
                                  in_=xc_src[:, 4 * b + 2:4 * b + 6, :])
            for k in range(6):
                q = nc.sync if k < 2 else nc.scalar
                q.dma_start(out=xat[4 + k][:],
                            in_=xa_src[:, 16 + 8 * k:26 + 8 * k, :])
                q.dma_start(out=xct[4 + k][:],
                            in_=xc_src[:, 18 + 8 * k:26 + 8 * k, :])

            bdv = consts.tile([128, N_BLKS, 32], bf16)
            nc.sync.dma_start(
                out=bdv, in_=bdv_d.ap().rearrange("p (b m) -> p b m", m=32))
            exv = consts.tile([32, N_BLKS, 128], bf16)
            nc.sync.dma_start(
                out=exv, in_=exv_d.ap().rearrange("p (b c) -> p b c", c=128))
            s2_sb = consts.tile([32, NG, BLK, 128], f32)
            nc.sync.dma_start(
                out=s2_sb,
                in_=s2_d.ap().rearrange("p (g r c) -> p g r c", r=BLK, c=128))
            eps_t = consts.tile([32, 1], f32)
            nc.gpsimd.memset(eps_t[:], 1e-9)

            out_v = out_d.ap().rearrange("p (r c) -> p r c", c=128)

            import contextlib
            loop_ctx = (tc.For_i(0, reps, 1,
                                 hint_engines=(mybir.EngineType.PE,
                                               mybir.EngineType.DVE,
                                               mybir.EngineType.Activation,
                                               mybir.EngineType.Pool,
                                               mybir.EngineType.SP))
                        if reps > 1 else contextlib.nullcontext())

            def conv_group(gi, b0, ng, psbs):
                y_ps = py.tile([8 * ng, BLK, 128], f32)
                for bp in range(ng):
                    blk = b0 + bp
                    if blk < 4:
                        ci, ro = blk, 0
                    else:
                        ci, ro = 4 + (blk - 4) // 2, 4 * ((blk - 4) % 2)
                    p_ps = pp.tile([128, BLK, 128], f32)
                    for k, j in enumerate(_MM_ORDER):
                        src, roff, coff = _MM_SLICES[j]
                        if src == 'XA':
                            xt, r = xat[ci], ro + roff
                        else:
                            xt, r = xct[ci], ro + roff - 2
                        if j == 12:  # K=64 single on lo partitions
                            lhsT = wlh[0:64, 1, :]
                            rhs = xt[0:64, r:r + BLK, coff:coff + 128]
                        else:
                            lhsT = (wlh[:, 0, :] if j == 0
                                    else wlr[:, j - 1, :])
                            rhs = xt[:, r:r + BLK, coff:coff + 128]
                        nc.tensor.matmul(p_ps[:], lhsT, rhs,
                                         start=(k == 0), stop=(k == 12))
                    psq = sq.tile([128, BLK, 128], bf16, tag="psq")
                    nc.scalar.activation(psq[:], p_ps[:], AF.Square)
                    p_sb = psb.tile([128, BLK, 128], bf16, tag="psb")
                    nc.scalar.activation(p_sb[:], p_ps[:], AF.Copy, bias=0.0)
                    psbs[blk] = p_sb
                    nc.tensor.matmul(y_ps[:], bdv[:, blk, 0:8 * ng], psq[:],
                                     start=(bp == 0), stop=(bp == ng - 1))
                return y_ps

            def factor(gi, ng, y_ps):
                # F = y/((1+y)*sqrt(y_raw+eps)), y = s^2*y_raw, on [8ng, 512]
                P = 8 * ng
                a_t = fac.tile([P, BLK, 128], f32, tag="a")
                nc.scalar.activation(a_t[:], y_ps[:], AF.Sqrt,
                                     bias=eps_t[0:P, :])
                y_t = fac.tile([P, BLK, 128], f32, tag="y")
                nc.vector.tensor_mul(y_t[:], y_ps[:], s2_sb[0:P, gi, :, :])
                y1_t = fac.tile([P, BLK, 128], f32, tag="y1")
                nc.scalar.activation(y1_t[:], y_t[:], AF.Copy, bias=1.0)
                b_t = fac.tile([P, BLK, 128], f32, tag="b")
                nc.vector.tensor_mul(b_t[:], a_t[:], y1_t[:])
                r_t = fac.tile([P, BLK, 128], f32, tag="r")
                nc.vector.reciprocal_approx_fast(r_t[:], b_t[:])
                F_t = ff.tile([P, BLK, 128], bf16, tag="F")
                nc.vector.tensor_mul(F_t[:], y_t[:], r_t[:])
                return F_t

            def expand(b0, ng, F_t, psbs):
                for bp in range(ng):
                    blk = b0 + bp
                    r0 = blk * BLK
                    fe_ps = pf.tile([128, BLK, 128], f32, tag="fe")
                    nc.tensor.matmul(fe_ps[:], exv[0:8 * ng, blk, :], F_t[:],
                                     start=True, stop=True)
                    v_t = vv.tile([128, BLK, 128], bf16, tag="v")
                    nc.vector.tensor_mul(v_t[:], psbs.pop(blk)[:], fe_ps[:])
                    nc.sync.dma_start(out=out_v[:, r0:r0 + BLK, :], in_=v_t[:])

            with loop_ctx:
                psbs = {}
                prev = None
                for gi, (b0, ng) in enumerate(GROUPS):
                    y_ps = conv_group(gi, b0, ng, psbs)
                    if prev is not None:
                        expand(prev[0], prev[1], prev[2], psbs)
                    prev = (b0, ng, factor(gi, ng, y_ps))
                expand(prev[0], prev[1], prev[2], psbs)

    nc.compile()
    return nc


def _prep_in_maps(u, W):
    import ml_dtypes
    bf = ml_dtypes.bfloat16
    x = u.reshape(N, CIN, H, W_SP)
    wl = _weight_tiles(W).reshape(128, 13 * 128).astype(bf)
    bdv = _bdv().astype(bf)
    exv = _exv().astype(bf)
    s2q = [_s2_groups(half) for half in range(2)]
    in_maps = []
    for core in range(N_CORES):
        n, half = core // 2, core % 2
        XA, XC = _inputs_core(x[n], half)
        in_maps.append({
            "xa": XA.reshape(128, XROWS * XCOLS).astype(bf),
            "xc": XC.reshape(128, XROWS * XCOLS).astype(bf),
            "wl": wl,
            "bdv": bdv,
            "exv": exv,
            "s2": s2q[half],
        })
    return in_maps


def run(u, W, trace=False):
    """Returns (out [N,T1,Z1,H,W] f32, BassKernelResults)."""
    from concourse.bass_utils import run_bass_kernel_spmd

    if "nc" not in _CACHE:
        _CACHE["nc"] = build_nc()
    nc = _CACHE["nc"]
    in_maps = _prep_in_maps(np.asarray(u, np.float32), np.asarray(W, np.float32))
    res = run_bass_kernel_spmd(nc, in_maps, list(range(N_CORES)), trace=trace)
    out = np.empty((N, T1, Z1, H, W_SP), np.float32)
    for core in range(N_CORES):
        n, half = core // 2, core % 2
        o = res.results[core]["out"].astype(np.float32).reshape(T1, Z1, ROWS, 128)
        out[n, :, :, half * 64:(half + 1) * 64, :] = o
    return out, res


def kernel(u, W):
    out, _ = run(u, W, trace=False)
    return out


# revision 34
# speedup vs baseline: 1.0492x; 1.0492x over previous
"""Trainium2 Bass kernel for nn_CapsuleLayer_9852654977072.

The reference module collapses mathematically: the routing loop's coupling
logits `b` stay zero (faithfully-reproduced bug in the original torch code),
so routing coefficients are a fixed spatial map r(h,w) = 1/(8*cnt(h,w)) where
cnt is the 5x5 box-count inside the image. The whole module is therefore:

    p = conv2d(u as [N,64,H,W], Wd as [128,64,5,5], pad=2) * s(h,w)
    v = squash_z1(p)   # groups of 16 channels
    out[n,t1,z1,h,w] = v

Device strategy (8 cores, SPMD): shard (batch n in 0..3) x (row-half in 0..1).
Each core computes all 128 output channels for 64 rows of one image.

Conv: inputs shipped as XA/XC [128, 68, 132] f32 whose partition halves hold u
shifted by (+0row,+1row) and (+2row+0col,+2row+1col) respectively, columns
padded by 2. Per 4-row block, 13 PSUM-accumulated fp32r matmuls (N=512, full
PE rate, self-loading weights) cover all 25 taps: 10 XA row-pairs +
2 XC col-pairs + 1 K=64 single.

Squash: per group of blocks ([4,4,4,2,2]), block-diagonal matmuls pack
m2 = sum_z1 q^2 for all (block, t1) pairs into one [8*ng, 512] PSUM tile.
The factor F = y/((1+y)*sqrt(y_raw+eps)), y = s^2*y_raw runs ONCE per group
on ACT/DVE in bf16 (no GpSimd), then expand matmuls broadcast F back to the
128 channels and v = p * F (bf16, shipped bf16 over HBM and upcast on host).
Expansion of group g-1 is emitted after the conv of group g so the PE queue
never stalls on the factor chain. Dummy matmuls at kernel start keep the PE
busy during the DMA lead-in so HAM stays un-throttled.
"""

import numpy as np

T0, Z0, T1, Z1, KK, PAD = 4, 16, 8, 16, 5, 2
N, H, W_SP = 4, 128, 128
CIN, COUT = T0 * Z0, T1 * Z1  # 64, 128
N_CORES = 8
ROWS = 64          # output rows per core
XROWS = 68         # input rows incl. halo
XCOLS = 132        # 128 + 2*PAD
BLK = 4            # output rows per block
N_BLKS = ROWS // BLK   # 16
GROUPS = [(0, 4), (4, 4), (8, 4), (12, 2), (14, 2)]  # (first block, n blocks)

# conv matmul j -> (source, row_off, col_off); weights match in _weight_tiles
_MM_SLICES = (
    [('XA', dy + 2, dx + 2) for dy in (-2, 0) for dx in (-2, -1, 0, 1, 2)]
    + [('XC', 2, 0), ('XC', 2, 2), ('XC', 2, 4)]
)

_MM_ORDER = [0, 12, 1, 2, 3, 4, 5, 6, 7, 8, 9, 10, 11]

_CACHE = {}


def _bp_of(blk):
    for b0, ng in GROUPS:
        if b0 <= blk < b0 + ng:
            return blk - b0
    raise ValueError(blk)


def _weight_tiles(W):
    Wd = W.transpose(1, 0, 2, 3, 4).reshape(COUT, CIN, KK, KK)
    wl = np.zeros((128, 13, 128), np.float32)  # [k, j, m]
    j = 0
    for dy in (-2, 0):
        for dx in (-2, -1, 0, 1, 2):
            wl[0:64, j, :] = Wd[:, :, dy + 2, dx + 2].T
            wl[64:128, j, :] = Wd[:, :, dy + 3, dx + 2].T
            j += 1
    for dx0 in (-2, 0):
        wl[0:64, j, :] = Wd[:, :, 4, dx0 + 2].T
        wl[64:128, j, :] = Wd[:, :, 4, dx0 + 3].T
        j += 1
    wl[0:64, j, :] = Wd[:, :, 4, 4].T  # single tap (2,2) on lo partitions
    return wl


def _inputs_core(x, half):
    """x: [64, H, W] one image channel-major. Returns XA, XC [128, 68, 132]."""
    base = half * 64 - 2
    XA = np.zeros((128, XROWS, XCOLS), np.float32)
    XC = np.zeros((128, XROWS, XCOLS), np.float32)

    def fill(dst, roff, c0, c1):
        lo, hi = max(0, -(base + roff)), min(XROWS, H - base - roff)
        dst[:, lo:hi, c0:c1] = x[:, base + roff + lo:base + roff + hi, :]

    fill(XA[0:64], 0, 2, 130)
    fill(XA[64:128], 1, 2, 130)
    fill(XC[0:64], 2, 2, 130)
    fill(XC[64:128], 2, 1, 129)
    return XA, XC


def _s2_groups(half):
    """[32, len(GROUPS), BLK*128] f32: s^2 at partition m=8*bp+t1
    (t1-replicated), group g, flat pos = (row-within-block, col)."""
    idx = np.arange(H)
    cnt = (np.minimum(idx + 2, H - 1) - np.maximum(idx - 2, 0) + 1).astype(np.float64)
    s = 1.0 / (8.0 * cnt[:, None] * cnt[None, :])  # [H, W]
    s2 = (s * s)[half * 64:(half + 1) * 64, :]     # [64, 128]
    out = np.zeros((32, len(GROUPS), BLK * 128), np.float64)
    for g, (b0, ng) in enumerate(GROUPS):
        for bp in range(ng):
            blk = b0 + bp
            rows = s2[blk * BLK:(blk + 1) * BLK, :].reshape(-1)  # [512]
            out[8 * bp:8 * bp + 8, g, :] = rows[None, :]
    return np.ascontiguousarray(
        out.astype(np.float32).reshape(32, len(GROUPS) * BLK * 128))


def _bdv():
    """[128, N_BLKS*32]: c=(t1,z1) -> partition m=8*bp(blk)+t1, sum over z1."""
    bd = np.zeros((128, N_BLKS, 32), np.float32)
    c = np.arange(128)
    for blk in range(N_BLKS):
        bd[c, blk, 8 * _bp_of(blk) + c // 16] = 1.0
    return np.ascontiguousarray(bd.reshape(128, N_BLKS * 32))


def _exv():
    """[32, N_BLKS*128]: partition p=8*bp(blk)+t1 -> channels c, c//16==t1."""
    ex = np.zeros((32, N_BLKS, 128), np.float32)
    c = np.arange(128)
    for blk in range(N_BLKS):
        ex[8 * _bp_of(blk) + c // 16, blk, c] = 1.0
    return np.ascontiguousarray(ex.reshape(32, N_BLKS * 128))


def build_nc(reps=1):
    import concourse.bass as bass
    import concourse.bacc as bacc
    import concourse.mybir as mybir
    import concourse.tile as tile

    f32 = mybir.dt.float32
    f32r = mybir.dt.float32r
    bf16 = mybir.dt.bfloat16
    AF = mybir.ActivationFunctionType
    NG = len(GROUPS)

    nc = bacc.Bacc(None, target_bir_lowering=False)
    xa_d = nc.dram_tensor("xa", [128, XROWS * XCOLS], bf16, kind="ExternalInput")
    xc_d = nc.dram_tensor("xc", [128, XROWS * XCOLS], bf16, kind="ExternalInput")
    wl_d = nc.dram_tensor("wl", [128, 13 * 128], bf16, kind="ExternalInput")
    bdv_d = nc.dram_tensor("bdv", [128, N_BLKS * 32], bf16, kind="ExternalInput")
    exv_d = nc.dram_tensor("exv", [32, N_BLKS * 128], bf16, kind="ExternalInput")
    s2_d = nc.dram_tensor("s2", [32, NG * BLK * 128], f32, kind="ExternalInput")
    out_d = nc.dram_tensor("out", [128, ROWS * 128], bf16, kind="ExternalOutput")

    with tile.TileContext(nc) as tc:
        with (
            tc.tile_pool(name="consts", bufs=1) as consts,
            tc.tile_pool(name="sq", bufs=5) as sq,
            tc.tile_pool(name="psb", bufs=9) as psb,
            tc.tile_pool(name="fac", bufs=2) as fac,
            tc.tile_pool(name="ff", bufs=2) as ff,
            tc.tile_pool(name="vv", bufs=3) as vv,
            tc.tile_pool(name="pp", bufs=4, space="PSUM") as pp,
            tc.tile_pool(name="py", bufs=2, space="PSUM") as py,
            tc.tile_pool(name="pf", bufs=2, space="PSUM") as pf,
        ):
            # PE pre-warm: dummy matmuls on a zeroed tile keep the PE busy
            # during the input-DMA lead-in so HAM un-throttles to 2.4 GHz
            # before the first real conv matmul.
            dum = consts.tile([128, 512], f32)
            nc.gpsimd.memset(dum[:], 0.0)
            dum_ps = pf.tile([128, BLK, 128], f32, tag="fe")
            for _ in range(13):
                nc.tensor.matmul(dum_ps[:], dum[:, 0:128].bitcast(f32r),
                                 dum[:].bitcast(f32r), start=True, stop=True)

            wl_src = wl_d.ap().rearrange("p (j m) -> p j m", m=128)
            wlh = consts.tile([128, 2, 128], bf16)
            nc.sync.dma_start(out=wlh[:, 0:1, :], in_=wl_src[:, 0:1, :])
            nc.sync.dma_start(out=wlh[:, 1:2, :], in_=wl_src[:, 12:13, :])
            wlr = consts.tile([128, 11, 128], bf16)

            # Input chunks as separate tiles (deps are whole-tile): tile 0
            # holds block 0's rows, tile i (1..8) covers blocks {2i-1, 2i}.
            xa_src = xa_d.ap().rearrange("p (r c) -> p r c", c=XCOLS)
            xc_src = xc_d.ap().rearrange("p (r c) -> p r c", c=XCOLS)
            # blocks 0-3: per-block tiles (fast arrival); blocks 4+: pairs
            xat = [consts.tile([128, 6, XCOLS], bf16, name=f"xat{b}")
                   for b in range(4)] + [
                consts.tile([128, 10, XCOLS], bf16, name=f"xap{k}")
                for k in range(6)]
            xct = [consts.tile([128, 4, XCOLS], bf16, name=f"xct{b}")
                   for b in range(4)] + [
                consts.tile([128, 8, XCOLS], bf16, name=f"xcp{k}")
                for k in range(6)]
            nc.sync.dma_start(out=xat[0][:], in_=xa_src[:, 0:6, :])
            nc.sync.dma_start(out=xct[0][:], in_=xc_src[:, 2:6, :])
            nc.sync.dma_start(out=wlr, in_=wl_src[:, 1:12, :])
            for b in range(1, 4):
                nc.sync.dma_start(out=xat[b][:],
                                  in_=xa_src[:, 4 * b:4 * b + 6, :])
                nc.sync.dma_start(out=xct[b][:],
                                  in_=xc_src[:, 4 * b + 2:4 * b + 6, :])
            for k in range(2):
                nc.sync.dma_start(out=xat[4 + k][:],
                                  in_=xa_src[:, 16 + 8 * k:26 + 8 * k, :])
                nc.sync.dma_start(out=xct[4 + k][:],
                                  in_=xc_src[:, 18 + 8 * k:26 + 8 * k, :])
            # gate the bulk chunks until the early critical bytes landed
            gate_t = consts.tile([1, 1], f32)
            nc.scalar.activation(gate_t[:], xat[3][0:1, 0, 0:2].bitcast(f32),
                                 AF.Copy, bias=0.0)
            for k in range(2, 6):
                nc.scalar.dma_start(out=xat[4 + k][:],
                                    in_=xa_src[:, 16 + 8 * k:26 + 8 * k, :])
                nc.scalar.dma_start(out=xct[4 + k][:],
                                    in_=xc_src[:, 18 + 8 * k:26 + 8 * k, :])

            bdv = consts.tile([128, N_BLKS, 32], bf16)
            nc.sync.dma_start(
                out=bdv, in_=bdv_d.ap().rearrange("p (b m) -> p b m", m=32))
            exv = consts.tile([32, N_BLKS, 128], bf16)
            nc.sync.dma_start(
                out=exv, in_=exv_d.ap().rearrange("p (b c) -> p b c", c=128))
            s2_sb = consts.tile([32, NG, BLK, 128], f32)
            nc.sync.dma_start(
                out=s2_sb,
                in_=s2_d.ap().rearrange("p (g r c) -> p g r c", r=BLK, c=128))
            eps_t = consts.tile([32, 1], f32)
            nc.gpsimd.memset(eps_t[:], 1e-9)

            out_v = out_d.ap().rearrange("p (r c) -> p r c", c=128)

            import contextlib
            loop_ctx = (tc.For_i(0, reps, 1,
                                 hint_engines=(mybir.EngineType.PE,
                                               mybir.EngineType.DVE,
                                               mybir.EngineType.Activation,
                                               mybir.EngineType.Pool,
                                               mybir.EngineType.SP))
                        if reps > 1 else contextlib.nullcontext())

            def conv_group(gi, b0, ng, psbs):
                y_ps = py.tile([8 * ng, BLK, 128], f32)
                psqs = []
                for bp in range(ng):
                    blk = b0 + bp
                    if blk < 4:
                        ci, ro = blk, 0
                    else:
                        ci, ro = 4 + (blk - 4) // 2, 4 * ((blk - 4) % 2)
                    p_ps = pp.tile([128, BLK, 128], f32)
                    for k, j in enumerate(_MM_ORDER):
                        src, roff, coff = _MM_SLICES[j]
                        if src == 'XA':
                            xt, r = xat[ci], ro + roff
                        else:
                            xt, r = xct[ci], ro + roff - 2
                        if j == 12:  # K=64 single on lo partitions
                            lhsT = wlh[0:64, 1, :]
                            rhs = xt[0:64, r:r + BLK, coff:coff + 128]
                        else:
                            lhsT = (wlh[:, 0, :] if j == 0
                                    else wlr[:, j - 1, :])
                            rhs = xt[:, r:r + BLK, coff:coff + 128]
                        nc.tensor.matmul(p_ps[:], lhsT, rhs,
                                         start=(k == 0), stop=(k == 12))
                    psq = sq.tile([128, BLK, 128], bf16, tag="psq")
                    nc.scalar.activation(psq[:], p_ps[:], AF.Square)
                    p_sb = psb.tile([128, BLK, 128], bf16, tag="psb")
                    nc.scalar.activation(p_sb[:], p_ps[:], AF.Copy, bias=0.0)
                    psbs[blk] = p_sb
                    psqs.append(psq)
                for bp in range(ng):
                    nc.tensor.matmul(y_ps[:], bdv[:, b0 + bp, 0:8 * ng],
                                     psqs[bp][:],
                                     start=(bp == 0), stop=(bp == ng - 1))
                return y_ps

            def factor(gi, ng, y_ps):
                # F = y/((1+y)*sqrt(y_raw+eps)), y = s^2*y_raw, on [8ng, 512]
                P = 8 * ng
                a_t = fac.tile([P, BLK, 128], f32, tag="a")
                nc.scalar.activation(a_t[:], y_ps[:], AF.Sqrt,
                                     bias=eps_t[0:P, :])
                y_t = fac.tile([P, BLK, 128], f32, tag="y")
                nc.vector.tensor_mul(y_t[:], y_ps[:], s2_sb[0:P, gi, :, :])
                y1_t = fac.tile([P, BLK, 128], f32, tag="y1")
                nc.scalar.activation(y1_t[:], y_t[:], AF.Copy, bias=1.0)
                b_t = fac.tile([P, BLK, 128], f32, tag="b")
                nc.vector.tensor_mul(b_t[:], a_t[:], y1_t[:])
                r_t = fac.tile([P, BLK, 128], f32, tag="r")
                nc.vector.reciprocal_approx_fast(r_t[:], b_t[:])
                F_t = ff.tile([P, BLK, 128], bf16, tag="F")
                nc.vector.tensor_mul(F_t[:], y_t[:], r_t[:])
                return F_t

            def expand(b0, ng, F_t, psbs):
                for bp in range(ng):
                    blk = b0 + bp
                    r0 = blk * BLK
                    fe_ps = pf.tile([128, BLK, 128], f32, tag="fe")
                    nc.tensor.matmul(fe_ps[:], exv[0:8 * ng, blk, :], F_t[:],
                                     start=True, stop=True)
                    v_t = vv.tile([128, BLK, 128], bf16, tag="v")
                    nc.vector.tensor_mul(v_t[:], psbs.pop(blk)[:], fe_ps[:])
                    nc.sync.dma_start(out=out_v[:, r0:r0 + BLK, :], in_=v_t[:])

            with loop_ctx:
                psbs = {}
                prev = None
                for gi, (b0, ng) in enumerate(GROUPS):
                    y_ps = conv_group(gi, b0, ng, psbs)
                    if prev is not None:
                        expand(prev[0], prev[1], prev[2], psbs)
                    prev = (b0, ng, factor(gi, ng, y_ps))
                expand(prev[0], prev[1], prev[2], psbs)

    nc.compile()
    return nc


def _prep_in_maps(u, W):
    import ml_dtypes
    bf = ml_dtypes.bfloat16
    x = u.reshape(N, CIN, H, W_SP)
    wl = _weight_tiles(W).reshape(128, 13 * 128).astype(bf)
    bdv = _bdv().astype(bf)
    exv = _exv().astype(bf)
    s2q = [_s2_groups(half) for half in range(2)]
    in_maps = []
    for core in range(N_CORES):
        n, half = core // 2, core % 2
        XA, XC = _inputs_core(x[n], half)
        in_maps.append({
            "xa": XA.reshape(128, XROWS * XCOLS).astype(bf),
            "xc": XC.reshape(128, XROWS * XCOLS).astype(bf),
            "wl": wl,
            "bdv": bdv,
            "exv": exv,
            "s2": s2q[half],
        })
    return in_maps


def run(u, W, trace=False):
    """Returns (out [N,T1,Z1,H,W] f32, BassKernelResults)."""
    from concourse.bass_utils import run_bass_kernel_spmd

    if "nc" not in _CACHE:
        _CACHE["nc"] = build_nc()
    nc = _CACHE["nc"]
    in_maps = _prep_in_maps(np.asarray(u, np.float32), np.asarray(W, np.float32))
    res = run_bass_kernel_spmd(nc, in_maps, list(range(N_CORES)), trace=trace)
    out = np.empty((N, T1, Z1, H, W_SP), np.float32)
    for core in range(N_CORES):
        n, half = core // 2, core % 2
        o = res.results[core]["out"].astype(np.float32).reshape(T1, Z1, ROWS, 128)
        out[n, :, :, half * 64:(half + 1) * 64, :] = o
    return out, res


def kernel(u, W):
    out, _ = run(u, W, trace=False)
    return out


# revision 35
# speedup vs baseline: 1.1017x; 1.0501x over previous
"""Trainium2 Bass kernel for nn_CapsuleLayer_9852654977072.

The reference module collapses mathematically: the routing loop's coupling
logits `b` stay zero (faithfully-reproduced bug in the original torch code),
so routing coefficients are a fixed spatial map r(h,w) = 1/(8*cnt(h,w)) where
cnt is the 5x5 box-count inside the image. The whole module is therefore:

    p = conv2d(u as [N,64,H,W], Wd as [128,64,5,5], pad=2) * s(h,w)
    v = squash_z1(p)   # groups of 16 channels
    out[n,t1,z1,h,w] = v

Device strategy (8 cores, SPMD): shard (batch n in 0..3) x (row-half in 0..1).
Each core computes all 128 output channels for 64 rows of one image.

Conv: inputs shipped as XA/XC [128, 68, 132] f32 whose partition halves hold u
shifted by (+0row,+1row) and (+2row+0col,+2row+1col) respectively, columns
padded by 2. Per 4-row block, 13 PSUM-accumulated fp32r matmuls (N=512, full
PE rate, self-loading weights) cover all 25 taps: 10 XA row-pairs +
2 XC col-pairs + 1 K=64 single.

Squash: per group of blocks ([4,4,4,2,2]), block-diagonal matmuls pack
m2 = sum_z1 q^2 for all (block, t1) pairs into one [8*ng, 512] PSUM tile.
The factor F = y/((1+y)*sqrt(y_raw+eps)), y = s^2*y_raw runs ONCE per group
on ACT/DVE in bf16 (no GpSimd), then expand matmuls broadcast F back to the
128 channels and v = p * F (bf16, shipped bf16 over HBM and upcast on host).
Expansion of group g-1 is emitted after the conv of group g so the PE queue
never stalls on the factor chain. Dummy matmuls at kernel start keep the PE
busy during the DMA lead-in so HAM stays un-throttled.
"""

import numpy as np

T0, Z0, T1, Z1, KK, PAD = 4, 16, 8, 16, 5, 2
N, H, W_SP = 4, 128, 128
CIN, COUT = T0 * Z0, T1 * Z1  # 64, 128
N_CORES = 8
ROWS = 64          # output rows per core
XROWS = 68         # input rows incl. halo
XCOLS = 132        # 128 + 2*PAD
BLK = 4            # output rows per block
N_BLKS = ROWS // BLK   # 16
GROUPS = [(0, 4), (4, 4), (8, 4), (12, 2), (14, 2)]  # (first block, n blocks)

# conv matmul j -> (source, row_off, col_off); weights match in _weight_tiles
_MM_SLICES = (
    [('XA', dy + 2, dx + 2) for dy in (-2, 0) for dx in (-2, -1, 0, 1, 2)]
    + [('XC', 2, 0), ('XC', 2, 2), ('XC', 2, 4)]
)

_MM_ORDER = [0, 12, 1, 2, 3, 4, 5, 6, 7, 8, 9, 10, 11]

_CACHE = {}


def _bp_of(blk):
    for b0, ng in GROUPS:
        if b0 <= blk < b0 + ng:
            return blk - b0
    raise ValueError(blk)


def _weight_tiles(W):
    Wd = W.transpose(1, 0, 2, 3, 4).reshape(COUT, CIN, KK, KK)
    wl = np.zeros((128, 13, 128), np.float32)  # [k, j, m]
    j = 0
    for dy in (-2, 0):
        for dx in (-2, -1, 0, 1, 2):
            wl[0:64, j, :] = Wd[:, :, dy + 2, dx + 2].T
            wl[64:128, j, :] = Wd[:, :, dy + 3, dx + 2].T
            j += 1
    for dx0 in (-2, 0):
        wl[0:64, j, :] = Wd[:, :, 4, dx0 + 2].T
        wl[64:128, j, :] = Wd[:, :, 4, dx0 + 3].T
        j += 1
    wl[0:64, j, :] = Wd[:, :, 4, 4].T  # single tap (2,2) on lo partitions
    return wl


def _inputs_core(x, half):
    """x: [64, H, W] one image channel-major. Returns XA, XC [128, 68, 132]."""
    base = half * 64 - 2
    XA = np.zeros((128, XROWS, XCOLS), np.float32)
    XC = np.zeros((128, XROWS, XCOLS), np.float32)

    def fill(dst, roff, c0, c1):
        lo, hi = max(0, -(base + roff)), min(XROWS, H - base - roff)
        dst[:, lo:hi, c0:c1] = x[:, base + roff + lo:base + roff + hi, :]

    fill(XA[0:64], 0, 2, 130)
    fill(XA[64:128], 1, 2, 130)
    fill(XC[0:64], 2, 2, 130)
    fill(XC[64:128], 2, 1, 129)
    return XA, XC


def _s2_groups(half):
    """[32, len(GROUPS), BLK*128] f32: s^2 at partition m=8*bp+t1
    (t1-replicated), group g, flat pos = (row-within-block, col)."""
    idx = np.arange(H)
    cnt = (np.minimum(idx + 2, H - 1) - np.maximum(idx - 2, 0) + 1).astype(np.float64)
    s = 1.0 / (8.0 * cnt[:, None] * cnt[None, :])  # [H, W]
    s2 = (s * s)[half * 64:(half + 1) * 64, :]     # [64, 128]
    out = np.zeros((32, len(GROUPS), BLK * 128), np.float64)
    for g, (b0, ng) in enumerate(GROUPS):
        for bp in range(ng):
            blk = b0 + bp
            rows = s2[blk * BLK:(blk + 1) * BLK, :].reshape(-1)  # [512]
            out[8 * bp:8 * bp + 8, g, :] = rows[None, :]
    return np.ascontiguousarray(
        out.astype(np.float32).reshape(32, len(GROUPS) * BLK * 128))


def _bdv():
    """[128, N_BLKS*32]: c=(t1,z1) -> partition m=8*bp(blk)+t1, sum over z1."""
    bd = np.zeros((128, N_BLKS, 32), np.float32)
    c = np.arange(128)
    for blk in range(N_BLKS):
        bd[c, blk, 8 * _bp_of(blk) + c // 16] = 1.0
    return np.ascontiguousarray(bd.reshape(128, N_BLKS * 32))


def _exv():
    """[32, N_BLKS*128]: partition p=8*bp(blk)+t1 -> channels c, c//16==t1."""
    ex = np.zeros((32, N_BLKS, 128), np.float32)
    c = np.arange(128)
    for blk in range(N_BLKS):
        ex[8 * _bp_of(blk) + c // 16, blk, c] = 1.0
    return np.ascontiguousarray(ex.reshape(32, N_BLKS * 128))


def build_nc(reps=1):
    import concourse.bass as bass
    import concourse.bacc as bacc
    import concourse.mybir as mybir
    import concourse.tile as tile

    f32 = mybir.dt.float32
    f32r = mybir.dt.float32r
    bf16 = mybir.dt.bfloat16
    AF = mybir.ActivationFunctionType
    NG = len(GROUPS)

    nc = bacc.Bacc(None, target_bir_lowering=False)
    xa_d = nc.dram_tensor("xa", [128, XROWS * XCOLS], bf16, kind="ExternalInput")
    xc_d = nc.dram_tensor("xc", [128, XROWS * XCOLS], bf16, kind="ExternalInput")
    wl_d = nc.dram_tensor("wl", [128, 13 * 128], bf16, kind="ExternalInput")
    bdv_d = nc.dram_tensor("bdv", [128, N_BLKS * 32], bf16, kind="ExternalInput")
    exv_d = nc.dram_tensor("exv", [32, N_BLKS * 128], bf16, kind="ExternalInput")
    s2_d = nc.dram_tensor("s2", [32, NG * BLK * 128], f32, kind="ExternalInput")
    out_d = nc.dram_tensor("out", [128, ROWS * 128], bf16, kind="ExternalOutput")

    with tile.TileContext(nc) as tc:
        with (
            tc.tile_pool(name="consts", bufs=1) as consts,
            tc.tile_pool(name="sq", bufs=5) as sq,
            tc.tile_pool(name="psb", bufs=9) as psb,
            tc.tile_pool(name="fac", bufs=2) as fac,
            tc.tile_pool(name="ff", bufs=2) as ff,
            tc.tile_pool(name="vv", bufs=3) as vv,
            tc.tile_pool(name="pp", bufs=4, space="PSUM") as pp,
            tc.tile_pool(name="py", bufs=2, space="PSUM") as py,
            tc.tile_pool(name="pf", bufs=2, space="PSUM") as pf,
        ):
            # PE pre-warm: dummy matmuls on a zeroed tile keep the PE busy
            # during the input-DMA lead-in so HAM un-throttles to 2.4 GHz
            # before the first real conv matmul.
            dum = consts.tile([128, 512], f32)
            nc.gpsimd.memset(dum[:], 0.0)
            dum_ps = pf.tile([128, BLK, 128], f32, tag="fe")
            for _ in range(17):
                nc.tensor.matmul(dum_ps[:], dum[:, 0:128].bitcast(f32r),
                                 dum[:].bitcast(f32r), start=True, stop=True)

            wl_src = wl_d.ap().rearrange("p (j m) -> p j m", m=128)
            wlh = consts.tile([128, 2, 128], bf16)
            nc.sync.dma_start(out=wlh[:, 0:1, :], in_=wl_src[:, 0:1, :])
            nc.sync.dma_start(out=wlh[:, 1:2, :], in_=wl_src[:, 12:13, :])
            wlr = consts.tile([128, 11, 128], bf16)

            # Input chunks as separate tiles (deps are whole-tile): tile 0
            # holds block 0's rows, tile i (1..8) covers blocks {2i-1, 2i}.
            xa_src = xa_d.ap().rearrange("p (r c) -> p r c", c=XCOLS)
            xc_src = xc_d.ap().rearrange("p (r c) -> p r c", c=XCOLS)
            # blocks 0-3: per-block tiles (fast arrival); blocks 4+: pairs
            xat = [consts.tile([128, 6, XCOLS], bf16, name=f"xat{b}")
                   for b in range(4)] + [
                consts.tile([128, 10, XCOLS], bf16, name=f"xap{k}")
                for k in range(6)]
            xct = [consts.tile([128, 4, XCOLS], bf16, name=f"xct{b}")
                   for b in range(4)] + [
                consts.tile([128, 8, XCOLS], bf16, name=f"xcp{k}")
                for k in range(6)]
            nc.sync.dma_start(out=xat[0][:], in_=xa_src[:, 0:6, :])
            nc.sync.dma_start(out=xct[0][:], in_=xc_src[:, 2:6, :])
            nc.sync.dma_start(out=wlr, in_=wl_src[:, 1:12, :])
            for b in range(1, 4):
                nc.sync.dma_start(out=xat[b][:],
                                  in_=xa_src[:, 4 * b:4 * b + 6, :])
                nc.sync.dma_start(out=xct[b][:],
                                  in_=xc_src[:, 4 * b + 2:4 * b + 6, :])
            for k in range(2):
                nc.sync.dma_start(out=xat[4 + k][:],
                                  in_=xa_src[:, 16 + 8 * k:26 + 8 * k, :])
                nc.sync.dma_start(out=xct[4 + k][:],
                                  in_=xc_src[:, 18 + 8 * k:26 + 8 * k, :])
            # gate the bulk chunks until the early critical bytes landed
            gate_t = consts.tile([1, 1], f32)
            nc.scalar.activation(gate_t[:], xat[3][0:1, 0, 0:2].bitcast(f32),
                                 AF.Copy, bias=0.0)
            for k in range(2, 6):
                nc.scalar.dma_start(out=xat[4 + k][:],
                                    in_=xa_src[:, 16 + 8 * k:26 + 8 * k, :])
                nc.scalar.dma_start(out=xct[4 + k][:],
                                    in_=xc_src[:, 18 + 8 * k:26 + 8 * k, :])

            bdv = consts.tile([128, N_BLKS, 32], bf16)
            nc.sync.dma_start(
                out=bdv, in_=bdv_d.ap().rearrange("p (b m) -> p b m", m=32))
            exv = consts.tile([32, N_BLKS, 128], bf16)
            nc.sync.dma_start(
                out=exv, in_=exv_d.ap().rearrange("p (b c) -> p b c", c=128))
            s2_sb = consts.tile([32, NG, BLK, 128], f32)
            nc.sync.dma_start(
                out=s2_sb,
                in_=s2_d.ap().rearrange("p (g r c) -> p g r c", r=BLK, c=128))
            eps_t = consts.tile([32, 1], f32)
            nc.gpsimd.memset(eps_t[:], 1e-9)

            out_v = out_d.ap().rearrange("p (r c) -> p r c", c=128)

            import contextlib
            loop_ctx = (tc.For_i(0, reps, 1,
                                 hint_engines=(mybir.EngineType.PE,
                                               mybir.EngineType.DVE,
                                               mybir.EngineType.Activation,
                                               mybir.EngineType.Pool,
                                               mybir.EngineType.SP))
                        if reps > 1 else contextlib.nullcontext())

            def conv_group(gi, b0, ng, psbs):
                y_ps = py.tile([8 * ng, BLK, 128], f32)
                psqs = []
                for bp in range(ng):
                    blk = b0 + bp
                    if blk < 4:
                        ci, ro = blk, 0
                    else:
                        ci, ro = 4 + (blk - 4) // 2, 4 * ((blk - 4) % 2)
                    p_ps = pp.tile([128, BLK, 128], f32)
                    for k, j in enumerate(_MM_ORDER):
                        src, roff, coff = _MM_SLICES[j]
                        if src == 'XA':
                            xt, r = xat[ci], ro + roff
                        else:
                            xt, r = xct[ci], ro + roff - 2
                        if j == 12:  # K=64 single on lo partitions
                            lhsT = wlh[0:64, 1, :]
                            rhs = xt[0:64, r:r + BLK, coff:coff + 128]
                        else:
                            lhsT = (wlh[:, 0, :] if j == 0
                                    else wlr[:, j - 1, :])
                            rhs = xt[:, r:r + BLK, coff:coff + 128]
                        nc.tensor.matmul(p_ps[:], lhsT, rhs,
                                         start=(k == 0), stop=(k == 12))
                    psq = sq.tile([128, BLK, 128], bf16, tag="psq")
                    nc.scalar.activation(psq[:], p_ps[:], AF.Square)
                    p_sb = psb.tile([128, BLK, 128], bf16, tag="psb")
                    nc.scalar.activation(p_sb[:], p_ps[:], AF.Copy, bias=0.0)
                    psbs[blk] = p_sb
                    psqs.append(psq)
                for bp in range(ng):
                    nc.tensor.matmul(y_ps[:], bdv[:, b0 + bp, 0:8 * ng],
                                     psqs[bp][:],
                                     start=(bp == 0), stop=(bp == ng - 1))
                return y_ps

            def factor(gi, ng, y_ps):
                # F = y/((1+y)*sqrt(y_raw+eps)), y = s^2*y_raw, on [8ng, 512]
                P = 8 * ng
                a_t = fac.tile([P, BLK, 128], f32, tag="a")
                nc.scalar.activation(a_t[:], y_ps[:], AF.Sqrt,
                                     bias=eps_t[0:P, :])
                y_t = fac.tile([P, BLK, 128], f32, tag="y")
                nc.vector.tensor_mul(y_t[:], y_ps[:], s2_sb[0:P, gi, :, :])
                y1_t = fac.tile([P, BLK, 128], f32, tag="y1")
                nc.scalar.activation(y1_t[:], y_t[:], AF.Copy, bias=1.0)
                b_t = fac.tile([P, BLK, 128], f32, tag="b")
                nc.vector.tensor_mul(b_t[:], a_t[:], y1_t[:])
                r_t = fac.tile([P, BLK, 128], f32, tag="r")
                nc.vector.reciprocal_approx_fast(r_t[:], b_t[:])
                F_t = ff.tile([P, BLK, 128], bf16, tag="F")
                nc.vector.tensor_mul(F_t[:], y_t[:], r_t[:])
                return F_t

            def expand(b0, ng, F_t, psbs):
                for bp in range(ng):
                    blk = b0 + bp
                    r0 = blk * BLK
                    fe_ps = pf.tile([128, BLK, 128], f32, tag="fe")
                    nc.tensor.matmul(fe_ps[:], exv[0:8 * ng, blk, :], F_t[:],
                                     start=True, stop=True)
                    v_t = vv.tile([128, BLK, 128], bf16, tag="v")
                    nc.vector.tensor_mul(v_t[:], psbs.pop(blk)[:], fe_ps[:])
                    nc.sync.dma_start(out=out_v[:, r0:r0 + BLK, :], in_=v_t[:])

            with loop_ctx:
                psbs = {}
                prev = None
                for gi, (b0, ng) in enumerate(GROUPS):
                    y_ps = conv_group(gi, b0, ng, psbs)
                    if prev is not None:
                        expand(prev[0], prev[1], prev[2], psbs)
                    prev = (b0, ng, factor(gi, ng, y_ps))
                expand(prev[0], prev[1], prev[2], psbs)

    nc.compile()
    return nc


def _prep_in_maps(u, W):
    import ml_dtypes
    bf = ml_dtypes.bfloat16
    x = u.reshape(N, CIN, H, W_SP)
    wl = _weight_tiles(W).reshape(128, 13 * 128).astype(bf)
    bdv = _bdv().astype(bf)
    exv = _exv().astype(bf)
    s2q = [_s2_groups(half) for half in range(2)]
    in_maps = []
    for core in range(N_CORES):
        n, half = core // 2, core % 2
        XA, XC = _inputs_core(x[n], half)
        in_maps.append({
            "xa": XA.reshape(128, XROWS * XCOLS).astype(bf),
            "xc": XC.reshape(128, XROWS * XCOLS).astype(bf),
            "wl": wl,
            "bdv": bdv,
            "exv": exv,
            "s2": s2q[half],
        })
    return in_maps


def run(u, W, trace=False):
    """Returns (out [N,T1,Z1,H,W] f32, BassKernelResults)."""
    from concourse.bass_utils import run_bass_kernel_spmd

    if "nc" not in _CACHE:
        _CACHE["nc"] = build_nc()
    nc = _CACHE["nc"]
    in_maps = _prep_in_maps(np.asarray(u, np.float32), np.asarray(W, np.float32))
    res = run_bass_kernel_spmd(nc, in_maps, list(range(N_CORES)), trace=trace)
    out = np.empty((N, T1, Z1, H, W_SP), np.float32)
    for core in range(N_CORES):
        n, half = core // 2, core % 2
        o = res.results[core]["out"].astype(np.float32).reshape(T1, Z1, ROWS, 128)
        out[n, :, :, half * 64:(half + 1) * 64, :] = o
    return out, res


def kernel(u, W):
    out, _ = run(u, W, trace=False)
    return out
